# revision 11
# baseline (speedup 1.0000x reference)
"""Trainium2 Bass kernel for nn_Decoder_91122026151952.

Math (reference collapses because LSTMCell state is zero every step):
    gates = x @ W_ih.T + (b_ih + b_hh)        # h0 == 0, W_hh unused
    i, f, g, o = split(gates, 4)              # f unused (c_prev == 0)
    c = sigmoid(i) * tanh(g)
    h = sigmoid(o) * tanh(c)                  # [B, T, H]
    out = softmax((h.reshape(B, T*H) @ W_out.T + b_out).reshape(B, 4, 10), -1)

Device formulation (fp16 operands, fp32 accumulation):
    AGO = sigmoid(x_aug @ W1aug)   where W1aug = [Wi.T | 2*Wg.T | Wo.T] plus a
    bias row matched to a ones-channel appended to x.  tanh(g) = 2*(G-0.5).
    u  = (G - 0.5) * A                        # == c/2, |u| < 0.5
    h  = (O * u) * (Q0 + Q1 * u*u)            # sigmoid(o)*tanh(2u), deg-3
                                              # odd minimax poly on |2u|<1
    logits.T [40, B] accumulate on PE over a flat 43200-deep contraction
    (h transposed on the DMA xbar in 1536-column chunks), bias via a rank-1
    ones matmul, final PE transpose + softmax on-chip.

Matmul1 structure (fp16, K=181 split 128+53): per timestep an A matmul
(K=128, full PE rows, start=True) and a B matmul (K=53, rows 0:53 of the
plain 128x128 array, accumulate) -- no array tiling, so no PE mode
switches and no same-bank row-tile concurrency (which is illegal).
N split 512 + 28: the 28-col tail accumulates into a per-superbatch PSUM
bank via the whole-bank has_written-clear trick (start=True only at
si==0).

Matmul2 runs as two concurrent accumulators on disjoint PE column groups
(tile_position col tiling), summed after a pair of PE transposes.

Elementwise on DVE is 3 scalar_tensor_tensor + 1 tensor_tensor per
superbatch; the only ACT work is the sigmoid itself.

Sharding: pure data parallel over batch (1024 -> 8 x 128).
Host prep: shard/cast/transpose/augment of inputs only.
"""

import numpy as np

B, T, H, OUT = 1024, 240, 180, 40
NCORES = 8
BC = B // NCORES            # 128 batches per core
G3 = 3 * H                  # 540 gate columns (i, g, o)
NBIG = 512                  # gate columns in the per-PB PSUM bank
NTAIL = G3 - NBIG           # 28 gate columns in the per-SB tail bank
KA = 128                    # features in the A (full-array) K-chunk
KB = 181 - KA               # 53 features (incl. ones/bias row) in B chunk
TB = 16                     # timesteps per input DMA batch
SB = 8                      # timesteps per DVE/ACT super-batch
PB = 2                      # timesteps per PSUM gates batch
TH = T * H                  # 43200 contraction depth of matmul2
SLOT = 1536                 # h columns per superbatch slot (SB*H=1440 + pad)
NSB = T // SB               # 30 superbatches
THP = NSB * SLOT            # 46080 padded contraction depth (360 slices)
NCH = THP // 128            # 360 th-slices
Q0 = 1.934069               # tanh(2u) ~ u*(Q0 + Q1*u^2), minimax |u|<0.5
Q1 = -1.659177

_CACHE = {}


def _build():
    import concourse.bass as bass
    import concourse.tile as tile
    from concourse import mybir

    f16 = mybir.dt.float16
    f32 = mybir.dt.float32
    ALU = mybir.AluOpType
    ACTF = mybir.ActivationFunctionType

    nc = bass.Bass("TRN2")

    xa = nc.dram_tensor("xa", [KA, T, BC], f16, kind="ExternalInput")
    xb = nc.dram_tensor("xb", [KB, T, BC], f16, kind="ExternalInput")
    w1a = nc.dram_tensor("w1a", [KA, G3], f16, kind="ExternalInput")
    w1b = nc.dram_tensor("w1b", [KB, G3], f16, kind="ExternalInput")
    w2 = nc.dram_tensor("w2", [128, NCH * OUT], f16, kind="ExternalInput")
    bout = nc.dram_tensor("bout", [1, OUT], f16, kind="ExternalInput")
    eye = nc.dram_tensor("eye", [OUT, OUT], f32, kind="ExternalInput")
    y = nc.dram_tensor("y", [BC, OUT], f32, kind="ExternalOutput")

    with tile.TileContext(nc) as tc:
        with (
            tc.tile_pool(name="consts", bufs=1) as consts,
            tc.tile_pool(name="xat", bufs=4) as xap,
            tc.tile_pool(name="xbt", bufs=4) as xbp,
            tc.tile_pool(name="work", bufs=3) as work,
            tc.tile_pool(name="fix", bufs=2) as fix,
            tc.tile_pool(name="htp", bufs=4) as htp,
            tc.tile_pool(name="gbig", bufs=2, space="PSUM") as gbig,
            tc.tile_pool(name="gtail", bufs=2, space="PSUM") as gtail,
            tc.tile_pool(name="mpsum", bufs=1, space="PSUM") as mpsum,
        ):
            # ---- constants; x prefetch first so the 3.7MB w2 doesn't sit
            # ahead of the first timesteps' inputs in the DMA FIFO ----
            w1a_sb = consts.tile([KA, G3], f16)
            nc.sync.dma_start(out=w1a_sb, in_=w1a[:, :])
            w1b_sb = consts.tile([KB, G3], f16)
            nc.sync.dma_start(out=w1b_sb, in_=w1b[:, :])

            NG = T // TB                       # 15 input load groups
            xq = []

            def load_group(g):
                t0 = g * TB
                at = xap.tile([KA, TB, BC], f16, tag="xa")
                nc.sync.dma_start(out=at, in_=xa[:, t0 : t0 + TB, :])
                bt = xbp.tile([KB, TB, BC], f16, tag="xb")
                nc.sync.dma_start(out=bt, in_=xb[:, t0 : t0 + TB, :])
                xq.append((at, bt))

            load_group(0)
            load_group(1)
            load_group(2)

            bout_sb = consts.tile([1, OUT], f16)
            nc.sync.dma_start(out=bout_sb, in_=bout[:, :])
            w2_sb = consts.tile([128, NCH * OUT], f16)
            nc.sync.dma_start(out=w2_sb, in_=w2[:, :])
            # identity for the final transposes: one copy based at partition
            # 0 (accumulator a) and one at partition 64 (accumulator b).
            eye_sb = consts.tile([104, OUT], f32)
            nc.sync.dma_start(out=eye_sb[0:OUT, :], in_=eye[:, :])
            nc.sync.dma_start(out=eye_sb[64 : 64 + OUT, :], in_=eye[:, :])
            ones_sb = consts.tile([1, BC], f16)
            nc.vector.memset(ones_sb, 1.0)

            # persistent h slots, pad region zeroed once (w2 pad rows are
            # zero so stale pad values would be harmless, but NaN*0=NaN --
            # keep them initialized).
            hs = []
            for k in range(3):
                hk = consts.tile([128, SLOT], f16, tag=f"hs{k}")
                nc.vector.memset(hk, 0.0)
                hs.append(hk)

            # matmul2 accumulators: logits.T on PE col groups 0 and 64
            mm2a = mpsum.tile([OUT, BC], f32)
            mm2b = mpsum.tile([64 + OUT, BC], f32)
            nc.tensor.matmul(
                mm2a, bout_sb, ones_sb,
                start=True, stop=False, skip_group_check=True,
            )

            ago = None
            gt = None
            for p in range(T // PB):
                t0 = PB * p
                # ---- input loads, TB timesteps at a time ----
                ti0 = t0 % TB
                if ti0 == 0:
                    g = t0 // TB
                    if g + 3 < NG:
                        load_group(g + 3)
                    xat, xbt = xq[g]

                sb, si0 = divmod(t0, SB)
                si1 = si0 + 1
                if si0 == 0:
                    ago = work.tile([128, SB, G3], f16, tag="ago")
                    gt = gtail.tile([128, SB, 64], f32, tag="gt")

                # ---- matmul1: per t an A matmul (K=128) then a B matmul
                # (K=53, rows 0:53, plain 128x128 mode) accumulating ----
                gps = gbig.tile([128, PB, NBIG], f32, tag="gates")
                for pi in range(PB):
                    ti = ti0 + pi
                    si = si0 + pi
                    nc.tensor.matmul(
                        gps[:, pi, :], xat[:, ti, :], w1a_sb[:, 0:NBIG],
                        start=True, stop=False, skip_group_check=True,
                    )
                    nc.tensor.matmul(
                        gt[:, si, 0:NTAIL], xat[:, ti, :], w1a_sb[:, NBIG:G3],
                        start=(si == 0), stop=False, skip_group_check=True,
                    )
                    nc.tensor.matmul(
                        gps[:, pi, :], xbt[:, ti, :], w1b_sb[:, 0:NBIG],
                        start=False, stop=True, skip_group_check=True,
                    )
                    nc.tensor.matmul(
                        gt[:, si, 0:NTAIL], xbt[:, ti, :], w1b_sb[:, NBIG:G3],
                        start=False, stop=(si == SB - 1),
                        skip_group_check=True,
                    )

                # ---- sigmoid over the PB-batch of big gates (PSUM -> SBUF) ----
                nc.scalar.activation(
                    out=ago[:, si0 : si1 + 1, 0:NBIG],
                    in_=gps[:, :, :],
                    func=ACTF.Sigmoid,
                )

                # ---- per super-batch: tail sigmoid, fused DVE chain, h ----
                if si1 == SB - 1:
                    nc.scalar.activation(
                        out=ago[:, :, NBIG:G3],
                        in_=gt[:, :, 0:NTAIL],
                        func=ACTF.Sigmoid,
                    )
                    # u = (G - 0.5) * A  == c/2
                    u = fix.tile([128, SB, H], f16, tag="u")
                    nc.vector.scalar_tensor_tensor(
                        u, ago[:, :, H : 2 * H], 0.5, ago[:, :, 0:H],
                        op0=ALU.subtract, op1=ALU.mult,
                    )
                    # pq = Q1 * u^2
                    pq = fix.tile([128, SB, H], f16, tag="pq")
                    nc.vector.scalar_tensor_tensor(
                        pq, u, Q1, u, op0=ALU.mult, op1=ALU.mult,
                    )
                    # m = O * u
                    m = fix.tile([128, SB, H], f16, tag="m")
                    nc.vector.tensor_tensor(
                        m, ago[:, :, 2 * H : G3], u, op=ALU.mult
                    )
                    # h = (pq + Q0) * m  into this superbatch's slot
                    hslot = hs[sb % 3]
                    nc.vector.scalar_tensor_tensor(
                        hslot[:, 0 : SB * H].rearrange("p (s h) -> p s h", s=SB),
                        pq, Q0, m, op0=ALU.add, op1=ALU.mult,
                    )
                    # one big xbar transpose per superbatch, then accumulate
                    htc = htp.tile([128, SLOT // 128, 128], f16, tag="htc")
                    nc.sync.dma_start(out=htc, in_=hslot, transpose=True)
                    for i in range(SLOT // 128):
                        sl = sb * (SLOT // 128) + i
                        if i % 2 == 0:
                            nc.tensor.matmul(
                                mm2a,
                                w2_sb[:, sl * OUT : (sl + 1) * OUT],
                                htc[:, i, :],
                                start=False, stop=(sl == NCH - 2),
                                skip_group_check=True,
                                tile_position=(0, 0),
                            )
                        else:
                            nc.tensor.matmul(
                                mm2b[64 : 64 + OUT, :],
                                w2_sb[:, sl * OUT : (sl + 1) * OUT],
                                htc[:, i, :],
                                start=(sl == 1), stop=(sl == NCH - 1),
                                skip_group_check=True,
                                tile_position=(0, 64),
                            )

            # ---- tail: transpose both logit accumulators, add, softmax ----
            facca = consts.tile([104, BC], f32)
            nc.vector.tensor_copy(facca[0:OUT, :], mm2a)
            nc.vector.tensor_copy(
                facca[64 : 64 + OUT, :], mm2b[64 : 64 + OUT, :]
            )
            tra = gbig.tile([BC, OUT], f32, tag="gates")
            nc.tensor.transpose(
                tra, facca[0:OUT, :], eye_sb[0:OUT, :], tile_position=(0, 0)
            )
            trb = gbig.tile([BC, OUT], f32, tag="gates")
            nc.tensor.transpose(
                trb,
                facca[64 : 64 + OUT, :],
                eye_sb[64 : 64 + OUT, :],
                tile_position=(64, 0),
            )
            trb_sb = consts.tile([BC, OUT], f32)
            nc.vector.tensor_copy(trb_sb, trb)
            lsum = consts.tile([BC, OUT], f32)
            nc.vector.tensor_tensor(lsum, tra, trb_sb, op=ALU.add)
            e_sb = consts.tile([BC, OUT], f32)
            nc.scalar.activation(out=e_sb, in_=lsum, func=ACTF.Exp)
            ssum = consts.tile([BC, 4], f32)
            nc.vector.tensor_reduce(
                ssum,
                e_sb.rearrange("p (g k) -> p g k", g=4),
                axis=mybir.AxisListType.X,
                op=ALU.add,
            )
            rinv = consts.tile([BC, 4], f32)
            nc.vector.reciprocal(rinv, ssum)
            y_sb = consts.tile([BC, OUT], f32)
            for g in range(4):
                nc.vector.tensor_scalar(
                    y_sb[:, g * 10 : (g + 1) * 10],
                    e_sb[:, g * 10 : (g + 1) * 10],
                    rinv[:, g : g + 1],
                    None,
                    op0=ALU.mult,
                )
            nc.sync.dma_start(out=y[:, :], in_=y_sb)

    _split_excess_waits(nc)
    return nc


def _split_excess_waits(nc):
    """walrus' per-instruction ISA structs have fewer sync-wait slots than
    Tile sometimes emits ("Too many sync wait commands"). For any instruction
    carrying >1 wait, insert EventSemaphore wait-carriers (one wait each)
    immediately before it on the same engine queue. The sequencer blocks on
    those first, then on the instruction's remaining wait — semantics are
    identical, no reordering is introduced."""
    import bass_rust
    import concourse.mybir as mybir

    n_new = 0
    for f in nc.m.functions:
        for blk in f.blocks:
            il = blk.instructions
            idx = 0
            while idx < len(il):
                ins = il[idx]
                si = getattr(ins, "sync_info", None)
                eng = getattr(ins, "engine", None)
                waits = list(si.on_wait) if si is not None else []
                if len(waits) >= 2 and eng is not None:
                    for w in waits[:-1]:
                        ev = mybir.InstEventSemaphore(
                            name=f"EVW-{n_new}", ins=[], outs=[]
                        )
                        n_new += 1
                        ev.engine = eng
                        ev.sync_info = bass_rust.SyncInfo(
                            on_wait=[w], on_update=[]
                        )
                        il.insert(idx, ev)
                        idx += 1
                    ins.sync_info = bass_rust.SyncInfo(
                        on_wait=[waits[-1]], on_update=list(si.on_update)
                    )
                idx += 1


def _prep_inputs(x, W_ih, b_ih, b_hh, W_out, b_out):
    """Host-side sharding prep: cast/transpose/augment. Returns per-core maps."""
    f16 = np.float16

    b = (b_ih + b_hh).astype(np.float32)
    Wi, Wg, Wo = W_ih[0:H], W_ih[2 * H : 3 * H], W_ih[3 * H : 4 * H]
    bi, bg, bo = b[0:H], b[2 * H : 3 * H], b[3 * H : 4 * H]
    W1 = np.concatenate([Wi.T, 2.0 * Wg.T, Wo.T], axis=1).astype(np.float32)
    brow = np.concatenate([bi, 2.0 * bg, bo])           # [540]
    w1a_np = W1[0:KA].astype(f16)                        # [128, 540]
    # B chunk: features 128..179 plus the bias row.
    w1b_np = np.zeros((KB, G3), dtype=f16)
    w1b_np[0 : KB - 1] = W1[KA:H].astype(f16)
    w1b_np[KB - 1] = brow.astype(f16)

    # W_out [40, 43200] -> per-superbatch padded th-major
    w2f = np.zeros((NSB, SLOT, OUT), dtype=np.float32)
    w2f[:, 0 : SB * H] = W_out.reshape(OUT, NSB, SB * H).transpose(1, 2, 0)
    w2t = (
        w2f.reshape(NCH, 128, OUT).transpose(1, 0, 2).reshape(128, NCH * OUT)
    ).astype(f16)

    boutq = b_out.astype(f16)[None, :]                   # [1, 40]
    eye = np.eye(OUT, dtype=np.float32)

    # x -> per-core xa [128, T, BC] (features 0:128) and xb [53, T, BC]
    # (features 128:180 + ones row for the bias).
    xs = x.reshape(NCORES, BC, T, H)
    in_maps = []
    for c in range(NCORES):
        xt = np.ascontiguousarray(xs[c].transpose(2, 1, 0))   # [H, T, BC]
        xa_np = xt[0:KA].astype(f16)
        xb_np = np.zeros((KB, T, BC), dtype=f16)
        xb_np[0 : KB - 1] = xt[KA:H].astype(f16)
        xb_np[KB - 1] = 1.0
        in_maps.append(
            {
                "xa": xa_np,
                "xb": xb_np,
                "w1a": w1a_np,
                "w1b": w1b_np,
                "w2": w2t,
                "bout": boutq,
                "eye": eye,
            }
        )
    return in_maps


def kernel(x, W_ih, W_hh, b_ih, b_hh, W_out, b_out, _bench=None):
    x = np.asarray(x, dtype=np.float32)
    W_ih = np.asarray(W_ih, dtype=np.float32)
    b_ih = np.asarray(b_ih, dtype=np.float32)
    b_hh = np.asarray(b_hh, dtype=np.float32)
    W_out = np.asarray(W_out, dtype=np.float32)
    b_out = np.asarray(b_out, dtype=np.float32)

    from concourse.bass_utils import run_bass_kernel_spmd

    if "nc" not in _CACHE:
        _CACHE["nc"] = _build()
    nc = _CACHE["nc"]

    in_maps = _prep_inputs(x, W_ih, b_ih, b_hh, W_out, b_out)
    kwargs = dict(_bench) if _bench else {}
    res = run_bass_kernel_spmd(nc, in_maps, core_ids=list(range(NCORES)), **kwargs)
    out = np.concatenate([r["y"] for r in res.results], axis=0)  # [1024, 40]
    if _bench is not None:
        _CACHE["last_result"] = res
    return out.reshape(B, 4, 10).astype(np.float32)


# revision 12
# speedup vs baseline: 1.9386x; 1.9386x over previous
"""Trainium2 Bass kernel for nn_Decoder_91122026151952.

Math (reference collapses because LSTMCell state is zero every step):
    gates = x @ W_ih.T + (b_ih + b_hh)        # h0 == 0, W_hh unused
    i, f, g, o = split(gates, 4)              # f unused (c_prev == 0)
    c = sigmoid(i) * tanh(g)
    h = sigmoid(o) * tanh(c)                  # [B, T, H]
    out = softmax((h.reshape(B, T*H) @ W_out.T + b_out).reshape(B, 4, 10), -1)

Device formulation (all-sigmoid matmul pass, fp8 operands, fp32 accum):
    AGO = sigmoid(x_aug @ W1aug)   where W1aug = [Wi.T | 2*Wg.T | Wo.T] plus a
    bias row matched to a ones-channel appended to x.  tanh(g) = 2*(G-0.5).
    u  = (G - 0.5) * A                        # == c/2, |u| < 0.5
    h  = (O * u) * (Q0 + Q1 * u*u)            # sigmoid(o)*tanh(2u) via deg-3
                                              # odd minimax poly (|2u| < 1)
    logits.T [40, B] accumulate on PE over a flat 43200-deep contraction
    (h transposed on the DMA xbar in 1536-column chunks), bias via a rank-1
    ones matmul, final PE transpose + softmax on-chip.

The PE in this environment runs at 1.2 GHz (1 col/cycle); fp8 DoubleRow
does the whole K=182 contraction in a single pass per column chunk, which
is the streaming floor (540 cols/t).  Per timestep: N=512 into a per-PB
PSUM bank + N=28 tail into a per-superbatch PSUM bank (whole-bank
has_written-clear trick: start=True only at si==0).

The elementwise chain runs on DVE only (ACT does just the sigmoids):
tensor_scalar at 4x and tensor_tensor at 2x fp16 modes
(scalar_tensor_tensor has only a 1x uop -- avoid).

Matmul2 runs as two concurrent accumulators on disjoint PE column groups
(tile_position col tiling), summed after a pair of PE transposes.  Each
superbatch's 12 matmul2 slices are DEFERRED by one superbatch so they
never head-of-line-block the PE queue while waiting on the h transpose
(this stall was ~3.5us per superbatch).

Sharding: pure data parallel over batch (1024 -> 8 x 128).
Host prep: shard/cast/transpose/augment of inputs only.
"""

import numpy as np

B, T, H, OUT = 1024, 240, 180, 40
NCORES = 8
BC = B // NCORES            # 128 batches per core
G3 = 3 * H                  # 540 gate columns (i, g, o)
G3P = 544                   # padded so the fp8 DoubleRow k-pair stride %16==0
NBIG = 512                  # gate columns in the per-PB PSUM bank
NTAIL = G3 - NBIG           # 28 gate columns in the per-SB tail bank
KP = 91                     # fp8 DoubleRow k-pairs: 182 virtual rows >= 181
W1SCALE = 16.0              # fp8 weight scale, undone by the sigmoid's affine
TB = 16                     # timesteps per input DMA batch
SB = 8                      # timesteps per DVE/ACT super-batch
PB = 2                      # timesteps per PSUM gates batch
TH = T * H                  # 43200 contraction depth of matmul2
SLOT = 1536                 # h columns per superbatch slot (SB*H=1440 + pad)
NSB = T // SB               # 30 superbatches
THP = NSB * SLOT            # 46080 padded contraction depth (360 slices)
NCH = THP // 128            # 360 th-slices
Q0 = 1.934069               # tanh(2u) ~ u*(Q0 + Q1*u^2), minimax |u|<0.5
Q1 = -1.659177

_CACHE = {}


def _build():
    import concourse.bass as bass
    import concourse.tile as tile
    from concourse import mybir

    f16 = mybir.dt.float16
    f32 = mybir.dt.float32
    f8 = mybir.dt.float8e4
    ALU = mybir.AluOpType
    ACTF = mybir.ActivationFunctionType
    DR = mybir.MatmulPerfMode.DoubleRow

    nc = bass.Bass("TRN2")

    xT = nc.dram_tensor("xT", [KP, T, 2 * BC], f8, kind="ExternalInput")
    w1 = nc.dram_tensor("w1", [KP, 2, G3P], f8, kind="ExternalInput")
    w2 = nc.dram_tensor("w2", [128, NCH * OUT], f16, kind="ExternalInput")
    bout = nc.dram_tensor("bout", [1, OUT], f16, kind="ExternalInput")
    eye = nc.dram_tensor("eye", [OUT, OUT], f32, kind="ExternalInput")
    y = nc.dram_tensor("y", [BC, OUT], f32, kind="ExternalOutput")

    with tile.TileContext(nc) as tc:
        with (
            tc.tile_pool(name="consts", bufs=1) as consts,
            tc.tile_pool(name="xtiles", bufs=4) as xtiles,
            tc.tile_pool(name="work", bufs=3) as work,
            tc.tile_pool(name="fix", bufs=2) as fix,
            tc.tile_pool(name="htp", bufs=4) as htp,
            tc.tile_pool(name="gbig", bufs=2, space="PSUM") as gbig,
            tc.tile_pool(name="gtail", bufs=2, space="PSUM") as gtail,
            tc.tile_pool(name="mpsum", bufs=1, space="PSUM") as mpsum,
        ):
            # ---- constants; x prefetch first so the 3.7MB w2 doesn't sit
            # ahead of the first timesteps' inputs in the DMA FIFO ----
            w1dr = consts.tile([KP, 2, G3P], f8)
            nc.sync.dma_start(out=w1dr, in_=w1[:, :, :])

            NG = T // TB                       # 15 input load groups
            xq = []

            def load_group(g):
                t0 = g * TB
                xt = xtiles.tile([KP, TB, 2 * BC], f8, tag="xt")
                nc.sync.dma_start(out=xt, in_=xT[:, t0 : t0 + TB, :])
                xq.append(xt)

            load_group(0)
            load_group(1)
            load_group(2)

            bout_sb = consts.tile([1, OUT], f16)
            nc.sync.dma_start(out=bout_sb, in_=bout[:, :])
            w2_sb = consts.tile([128, NCH * OUT], f16)
            nc.sync.dma_start(out=w2_sb, in_=w2[:, :])
            # identity for the final transposes: one copy based at partition
            # 0 (accumulator a) and one at partition 64 (accumulator b).
            eye_sb = consts.tile([104, OUT], f32)
            nc.sync.dma_start(out=eye_sb[0:OUT, :], in_=eye[:, :])
            nc.sync.dma_start(out=eye_sb[64 : 64 + OUT, :], in_=eye[:, :])
            ones_sb = consts.tile([1, BC], f16)
            nc.vector.memset(ones_sb, 1.0)

            # persistent h slots; pad region [SB*H:SLOT] zeroed once (the
            # matching w2 rows are zero, but stale NaN * 0 = NaN -- so the
            # pads must be initialized; they are never written again).
            hs = []
            for k in range(3):
                hk = consts.tile([128, SLOT], f16, tag=f"hs{k}")
                nc.vector.memset(hk, 0.0)
                hs.append(hk)

            # matmul2 accumulators: logits.T on PE col groups 0 and 64
            mm2a = mpsum.tile([OUT, BC], f32)
            mm2b = mpsum.tile([64 + OUT, BC], f32)
            nc.tensor.matmul(
                mm2a, bout_sb, ones_sb,
                start=True, stop=False, skip_group_check=True,
            )

            def mm2_batch(sb, htc):
                """Emit the 12 matmul2 slice-accumulations for superbatch sb."""
                for i in range(SLOT // 128):
                    sl = sb * (SLOT // 128) + i
                    if i % 2 == 0:
                        nc.tensor.matmul(
                            mm2a,
                            w2_sb[:, sl * OUT : (sl + 1) * OUT],
                            htc[:, i, :],
                            start=False, stop=(sl == NCH - 2),
                            skip_group_check=True,
                            tile_position=(0, 0),
                        )
                    else:
                        nc.tensor.matmul(
                            mm2b[64 : 64 + OUT, :],
                            w2_sb[:, sl * OUT : (sl + 1) * OUT],
                            htc[:, i, :],
                            start=(sl == 1), stop=(sl == NCH - 1),
                            skip_group_check=True,
                            tile_position=(0, 64),
                        )

            pending = None                     # (sb, htc) awaiting matmul2
            ago = None
            gt = None
            for t in range(T):
                # ---- input loads, TB timesteps at a time ----
                ti = t % TB
                if ti == 0:
                    g = t // TB
                    if g + 3 < NG:
                        load_group(g + 3)
                    xt = xq[g]

                sb, si = divmod(t, SB)
                if si == 0:
                    ago = work.tile([128, SB, G3], f16, tag="ago")
                    gt = gtail.tile([128, SB, 64], f32, tag="gt")

                # ---- matmul1 (fp8 DoubleRow, one K-chunk): gates for t ----
                xdr = xt[:, ti, :].rearrange("p (k b) -> p k b", k=2)
                pi = t % PB
                if pi == 0:
                    gps = gbig.tile([128, PB, NBIG], f32, tag="gates")
                nc.tensor.matmul(
                    gps[:, pi, :],
                    xdr,
                    w1dr[:, :, 0:NBIG],
                    start=True, stop=True, perf_mode=DR,
                )
                nc.tensor.matmul(
                    gt[:, si, 0:NTAIL],
                    xdr,
                    w1dr[:, :, NBIG:G3],
                    start=(si == 0), stop=(si == SB - 1),
                    perf_mode=DR, skip_group_check=True,
                )

                # ---- sigmoid over the PB-batch of big gates (PSUM -> SBUF) ----
                if pi == PB - 1:
                    nc.scalar.activation(
                        out=ago[:, si - (PB - 1) : si + 1, 0:NBIG],
                        in_=gps[:, :, :],
                        func=ACTF.Sigmoid,
                        scale=1.0 / W1SCALE,
                    )

                # ---- per super-batch: tail sigmoid, DVE chain, h, transpose ----
                if si == SB - 1:
                    nc.scalar.activation(
                        out=ago[:, :, NBIG:G3],
                        in_=gt[:, :, 0:NTAIL],
                        func=ACTF.Sigmoid,
                        scale=1.0 / W1SCALE,
                    )
                    # g2 = G - 0.5  (== tanh(g)/2)
                    g2 = fix.tile([128, SB, H], f16, tag="g2")
                    nc.vector.tensor_scalar(
                        g2, ago[:, :, H : 2 * H], 0.5, None, op0=ALU.subtract
                    )
                    # u = A * g2  (== c/2)
                    u = fix.tile([128, SB, H], f16, tag="u")
                    nc.vector.tensor_tensor(u, ago[:, :, 0:H], g2, op=ALU.mult)
                    # s = u * u
                    s = fix.tile([128, SB, H], f16, tag="s")
                    nc.vector.tensor_tensor(s, u, u, op=ALU.mult)
                    # w = Q1*s + Q0   (tanh(2u)/u)
                    w = fix.tile([128, SB, H], f16, tag="w")
                    nc.vector.tensor_scalar(
                        w, s, Q1, Q0, op0=ALU.mult, op1=ALU.add
                    )
                    # m = O * u
                    m = fix.tile([128, SB, H], f16, tag="m")
                    nc.vector.tensor_tensor(
                        m, ago[:, :, 2 * H : G3], u, op=ALU.mult
                    )
                    # h = w * m  into this superbatch's slot
                    hslot = hs[sb % 3]
                    nc.vector.tensor_tensor(
                        hslot[:, 0 : SB * H].rearrange("p (s h) -> p s h", s=SB),
                        w,
                        m,
                        op=ALU.mult,
                    )
                    # one big xbar transpose per superbatch; matmul2 for this
                    # superbatch is deferred until the NEXT superbatch's mm1
                    # stream so it never blocks the PE queue.
                    htc = htp.tile([128, SLOT // 128, 128], f16, tag="htc")
                    nc.sync.dma_start(out=htc, in_=hslot, transpose=True)
                    if pending is not None:
                        mm2_batch(*pending)
                    pending = (sb, htc)

            if pending is not None:
                mm2_batch(*pending)

            # ---- tail: transpose both logit accumulators, add, softmax ----
            facca = consts.tile([104, BC], f32)
            nc.vector.tensor_copy(facca[0:OUT, :], mm2a)
            nc.vector.tensor_copy(
                facca[64 : 64 + OUT, :], mm2b[64 : 64 + OUT, :]
            )
            tra = gbig.tile([BC, OUT], f32, tag="gates")
            nc.tensor.transpose(
                tra, facca[0:OUT, :], eye_sb[0:OUT, :], tile_position=(0, 0)
            )
            trb = gbig.tile([BC, OUT], f32, tag="gates")
            nc.tensor.transpose(
                trb,
                facca[64 : 64 + OUT, :],
                eye_sb[64 : 64 + OUT, :],
                tile_position=(64, 0),
            )
            trb_sb = consts.tile([BC, OUT], f32)
            nc.vector.tensor_copy(trb_sb, trb)
            lsum = consts.tile([BC, OUT], f32)
            nc.vector.tensor_tensor(lsum, tra, trb_sb, op=ALU.add)
            e_sb = consts.tile([BC, OUT], f32)
            nc.scalar.activation(out=e_sb, in_=lsum, func=ACTF.Exp)
            ssum = consts.tile([BC, 4], f32)
            nc.vector.tensor_reduce(
                ssum,
                e_sb.rearrange("p (g k) -> p g k", g=4),
                axis=mybir.AxisListType.X,
                op=ALU.add,
            )
            rinv = consts.tile([BC, 4], f32)
            nc.vector.reciprocal(rinv, ssum)
            y_sb = consts.tile([BC, OUT], f32)
            for g in range(4):
                nc.vector.tensor_scalar(
                    y_sb[:, g * 10 : (g + 1) * 10],
                    e_sb[:, g * 10 : (g + 1) * 10],
                    rinv[:, g : g + 1],
                    None,
                    op0=ALU.mult,
                )
            nc.sync.dma_start(out=y[:, :], in_=y_sb)

    _split_excess_waits(nc)
    return nc


def _split_excess_waits(nc):
    """walrus' per-instruction ISA structs have fewer sync-wait slots than
    Tile sometimes emits ("Too many sync wait commands"). For any instruction
    carrying >1 wait, insert EventSemaphore wait-carriers (one wait each)
    immediately before it on the same engine queue. The sequencer blocks on
    those first, then on the instruction's remaining wait — semantics are
    identical, no reordering is introduced."""
    import bass_rust
    import concourse.mybir as mybir

    n_new = 0
    for f in nc.m.functions:
        for blk in f.blocks:
            il = blk.instructions
            idx = 0
            while idx < len(il):
                ins = il[idx]
                si = getattr(ins, "sync_info", None)
                eng = getattr(ins, "engine", None)
                waits = list(si.on_wait) if si is not None else []
                if len(waits) >= 2 and eng is not None:
                    for w in waits[:-1]:
                        ev = mybir.InstEventSemaphore(
                            name=f"EVW-{n_new}", ins=[], outs=[]
                        )
                        n_new += 1
                        ev.engine = eng
                        ev.sync_info = bass_rust.SyncInfo(
                            on_wait=[w], on_update=[]
                        )
                        il.insert(idx, ev)
                        idx += 1
                    ins.sync_info = bass_rust.SyncInfo(
                        on_wait=[waits[-1]], on_update=list(si.on_update)
                    )
                idx += 1


def _prep_inputs(x, W_ih, b_ih, b_hh, W_out, b_out):
    """Host-side sharding prep: cast/transpose/augment. Returns per-core maps."""
    import ml_dtypes

    f16 = np.float16
    f8 = ml_dtypes.float8_e4m3fn
    b = (b_ih + b_hh).astype(np.float32)
    Wi, Wg, Wo = W_ih[0:H], W_ih[2 * H : 3 * H], W_ih[3 * H : 4 * H]
    bi, bg, bo = b[0:H], b[2 * H : 3 * H], b[3 * H : 4 * H]
    W1 = np.concatenate([Wi.T, 2.0 * Wg.T, Wo.T], axis=1).astype(np.float32)
    brow = np.concatenate([bi, 2.0 * bg, bo])[None, :]
    w1a = np.concatenate([W1, brow], axis=0) * W1SCALE       # [181, 540]
    # DoubleRow pack: virtual row r -> (r // 2, r % 2); rows 181 zero-padded
    w1p = np.zeros((2 * KP, G3P), dtype=np.float32)
    w1p[0 : H + 1, 0:G3] = w1a
    w1q = np.ascontiguousarray(w1p.reshape(KP, 2, G3P)).astype(f8)

    # W_out [40, 43200] -> per-superbatch padded th-major
    w2f = np.zeros((NSB, SLOT, OUT), dtype=np.float32)
    w2f[:, 0 : SB * H] = W_out.reshape(OUT, NSB, SB * H).transpose(1, 2, 0)
    w2t = (
        w2f.reshape(NCH, 128, OUT).transpose(1, 0, 2).reshape(128, NCH * OUT)
    ).astype(f16)

    boutq = b_out.astype(f16)[None, :]                       # [1, 40]
    eye = np.eye(OUT, dtype=np.float32)

    # x -> per-core [KP, T, 2*BC] fp8: channel c at (c//2, :, (c%2)*BC + b),
    # ones channel at 180 -> (90, :, 0*BC + b), channel 181 zero pad
    xs = x.reshape(NCORES, BC, T, H)
    in_maps = []
    for c in range(NCORES):
        xc = np.zeros((2 * KP, T, BC), dtype=np.float32)
        xc[0:H] = xs[c].transpose(2, 1, 0)                   # [H, T, BC]
        xc[H] = 1.0
        xq = np.ascontiguousarray(
            xc.reshape(KP, 2, T, BC).transpose(0, 2, 1, 3).reshape(KP, T, 2 * BC)
        ).astype(f8)
        in_maps.append(
            {
                "xT": xq,
                "w1": w1q,
                "w2": w2t,
                "bout": boutq,
                "eye": eye,
            }
        )
    return in_maps


def kernel(x, W_ih, W_hh, b_ih, b_hh, W_out, b_out, _bench=None):
    x = np.asarray(x, dtype=np.float32)
    W_ih = np.asarray(W_ih, dtype=np.float32)
    b_ih = np.asarray(b_ih, dtype=np.float32)
    b_hh = np.asarray(b_hh, dtype=np.float32)
    W_out = np.asarray(W_out, dtype=np.float32)
    b_out = np.asarray(b_out, dtype=np.float32)

    from concourse.bass_utils import run_bass_kernel_spmd

    if "nc" not in _CACHE:
        _CACHE["nc"] = _build()
    nc = _CACHE["nc"]

    in_maps = _prep_inputs(x, W_ih, b_ih, b_hh, W_out, b_out)
    kwargs = dict(_bench) if _bench else {}
    res = run_bass_kernel_spmd(nc, in_maps, core_ids=list(range(NCORES)), **kwargs)
    out = np.concatenate([r["y"] for r in res.results], axis=0)  # [1024, 40]
    if _bench is not None:
        _CACHE["last_result"] = res
    return out.reshape(B, 4, 10).astype(np.float32)
